# revision 34
# baseline (speedup 1.0000x reference)
"""Trainium2 Bass kernel for nn_DomainMapper (segment_reduce + tiny MLP).

Computation (matches the reference):
    sums[s]   = sum of x rows with label s          [32, 640]
    counts[s] = number of rows with label s         [32]
    feats     = sums / counts
    h         = relu(feats @ W1 + b1)               [32, 256]
    logits    = h @ W2 + b2                         [32, 32]
    probs     = softmax(logits, axis=-1)
    returns (probs, arange(32))

Strategy: data-parallel over 8 NeuronCores, two launches.

Stage 1 (SPMD x8): each core streams its 16384-row shard of x once (the
memory-bound part) and computes local segment sums + counts via one-hot
matmuls on the TensorEngine, writing a [32, 641] partial. On-chip
collectives were measured at 30-120us of exposed latency for this 82KB
payload (entry-barrier/launch-skew dominated), so the 8 partials are
gathered and summed on the host instead (8 x 82KB of shard glue).

Stage 2 (1 core): the reduced sums+counts go back to the device; the tiny
MLP + softmax runs in a second kernel and produces probs.

fp32 matmuls run at 1/4 PE rate on trn2, so x is shipped as a bf16 hi/lo
pair (x == hi + lo to ~16 mantissa bits): same 4 bytes/element of HBM
traffic as fp32, but the segment-sum matmuls run at full bf16 rate while
accumulating in fp32 PSUM. Counts are accumulated on the
VectorEngine (acc += onehot per subtile) and column-summed by one tiny
matmul at the end.
"""

import ml_dtypes
import numpy as np

import concourse.bass as bass
import concourse.bacc as bacc
import concourse.mybir as mybir
import concourse.tile as tile
from concourse.bass_utils import run_bass_kernel_spmd

N_CORES = 8
N, D, H, S = 131072, 640, 256, 32
XW = 640             # x hi/lo row: 2*640*2 = 2560 B, naturally 64B-aligned
                     # (counts come from a DVE one-hot accumulator instead of
                     # an appended ones column, so no pad bytes are read)
ROWS = N // N_CORES  # rows per core
P = 128              # partitions / rows per matmul subtile
KC1 = D // P         # 5 contraction chunks for feats @ W1
KC2 = H // P         # 2 contraction chunks for h @ W2

PROFILE = False
LAST_EXEC_NS = None
LAST_MAIN_NS = None
LAST_MLP_NS = None
LAST_RESULTS = None

_nc_cache = {}


def build_main_nc(rows=ROWS, g=4):
    """Stage-1 graph (SPMD x8): local segment sums + counts -> part[32, XW]."""
    T = rows // P            # number of 128-row subtiles
    assert T % g == 0
    f32 = mybir.dt.float32
    bf16 = mybir.dt.bfloat16
    ALU = mybir.AluOpType

    nc = bacc.Bacc("TRN2", target_bir_lowering=False, debug=False,
                   num_devices=N_CORES)

    xhl = nc.dram_tensor("xhl", [rows, 2, XW], bf16, kind="ExternalInput").ap()
    labt = nc.dram_tensor("labt", [P, T], f32, kind="ExternalInput").ap()
    iota = nc.dram_tensor("iota", [P, S], f32, kind="ExternalInput").ap()
    part_out = nc.dram_tensor("part", [S, D + 1], f32, kind="ExternalOutput").ap()

    with tile.TileContext(nc) as tc:
        with (
            tc.tile_pool(name="const", bufs=1) as cpool,
            tc.tile_pool(name="xload", bufs=2) as xpool,
            tc.tile_pool(name="oh", bufs=6) as ohpool,
            tc.tile_pool(name="acc", bufs=1, space=bass.MemorySpace.PSUM) as apool,
            tc.tile_pool(name="small", bufs=1) as spool,
        ):
            # labt/iota ride the scalar ring so the sync ring's first job is
            # x chunk 0 (first matmul fires ~3us earlier)
            labt_sb = cpool.tile([P, T], f32)
            nc.scalar.dma_start(labt_sb[:], labt[:])
            iota_sb = cpool.tile([P, S], f32)
            nc.scalar.dma_start(iota_sb[:], iota[:])
            acc_sb = cpool.tile([P, S], f32)   # per-partition one-hot sums
            nc.vector.memset(acc_sb[:], 0.0)
            ones_col = cpool.tile([P, 1], f32)
            nc.vector.memset(ones_col[:], 1.0)

            # ---- local segment sums: psum += onehot.T @ [x_hi; x_lo] ----
            psumA = apool.tile([S, 512], f32)       # x cols 0:512
            psumB = apool.tile([S, XW - 512], f32)  # x cols 512:640
            psumC = apool.tile([S, 1], f32)         # counts

            # Chunk plan: tiny leading DMAs so the PE starts ~20us earlier
            # (a big first DMA shares SDMA engines with the other queues and
            # completes late), then steady g-subtile groups.
            chunks = []
            lead = [1, 1, 2]
            if T > sum(lead) and (T - sum(lead)) % g == 0:
                t0 = 0
                for c in lead:
                    chunks.append((t0, c))
                    t0 += c
                while t0 < T:
                    chunks.append((t0, g))
                    t0 += g
            else:
                chunks = [(t0, g) for t0 in range(0, T, g)]

            xc = xhl.rearrange("(t p) two d -> p t two d", p=P)
            for ci, (t0, c) in enumerate(chunks):
                xt = xpool.tile([P, c, 2, XW], bf16, tag=f"xt{c}",
                                name=f"xt_{t0}", bufs=10 if c == g else 2)
                # alternate the two HWDGE rings (sync / scalar engines)
                dma_eng = nc.sync if ci % 2 == 0 else nc.scalar
                dma_eng.dma_start(xt[:], xc[:, t0:t0 + c])
                for j in range(c):
                    t = t0 + j
                    oh = ohpool.tile([P, S], bf16)
                    nc.vector.tensor_scalar(
                        oh[:], iota_sb[:], labt_sb[:, t:t + 1], None, ALU.is_equal)
                    nc.vector.tensor_tensor(acc_sb[:], acc_sb[:], oh[:], ALU.add)
                    first = (t == 0)
                    last = (t == T - 1)
                    # hi pass then lo pass share the same stationary one-hot
                    nc.tensor.matmul(psumA[:], oh[:], xt[:, j, 0, 0:512],
                                     start=first, stop=False)
                    nc.tensor.matmul(psumB[:], oh[:], xt[:, j, 0, 512:XW],
                                     start=first, stop=False)
                    nc.tensor.matmul(psumA[:], oh[:], xt[:, j, 1, 0:512],
                                     start=False, stop=last)
                    nc.tensor.matmul(psumB[:], oh[:], xt[:, j, 1, 512:XW],
                                     start=False, stop=last)

            # counts[s] = sum_p acc[p, s] via a single 1-column matmul
            nc.tensor.matmul(psumC[:], acc_sb[:], ones_col[:])
            part = spool.tile([S, D + 1], f32)
            nc.vector.tensor_copy(part[:, 0:512], psumA[:])
            nc.vector.tensor_copy(part[:, 512:D], psumB[:])
            nc.vector.tensor_copy(part[:, D:D + 1], psumC[:])
            nc.sync.dma_start(part_out[:], part[:])

    nc.compile()
    return nc


# Stage-2 packed input layout: one [128, MLP_W] f32 buffer holding everything.
# sums are packed TRANSPOSED ([p, k, s] = sums[s, k*128+p]) so the feats @ W1
# contraction needs no on-device transposes; 1/counts is applied after that
# matmul (diag(1/c) @ (sums@W1)) via the ReLU activation's per-partition
# scale, with counts (x) b1 added in PSUM by a counts-row matmul so the bias
# comes out right: relu((sums@W1 + c (x) b1) / c) == relu(feats@W1 + b1).
MLP_W1 = 0                       # [128, KC1*H]
MLP_W2 = MLP_W1 + KC1 * H        # [128, KC2*S]
MLP_B1 = MLP_W2 + KC2 * S        # [1, H] (partition 0)
MLP_B2 = MLP_B1 + H              # [1, S] (partition 0)
MLP_CROW = MLP_B2 + S            # [1, S] counts row (partition 0)
MLP_TT = MLP_CROW + S            # [128, KC1*S] sums transposed
MLP_CNT = MLP_TT + KC1 * S       # [32, 1] counts column
MLP_ID = MLP_CNT + 1             # [32, S] identity (partitions 0:32)
MLP_W = MLP_ID + S


def pack_mlp_input(tot, W1, b1, W2, b2):
    buf = np.zeros((P, MLP_W), np.float32)
    w1 = np.asarray(W1, dtype=np.float32).reshape(KC1, P, H)
    buf[:, MLP_W1:MLP_W1 + KC1 * H] = w1.transpose(1, 0, 2).reshape(P, KC1 * H)
    w2 = np.asarray(W2, dtype=np.float32).reshape(KC2, P, S)
    buf[:, MLP_W2:MLP_W2 + KC2 * S] = w2.transpose(1, 0, 2).reshape(P, KC2 * S)
    buf[0, MLP_B1:MLP_B1 + H] = np.asarray(b1, dtype=np.float32).ravel()
    buf[0, MLP_B2:MLP_B2 + S] = np.asarray(b2, dtype=np.float32).ravel()
    counts = tot[:, D]
    buf[0, MLP_CROW:MLP_CROW + S] = counts
    tt = tot[:, :D].T.reshape(KC1, P, S)  # [k, p, s] = sums[s, k*128+p]
    buf[:, MLP_TT:MLP_TT + KC1 * S] = tt.transpose(1, 0, 2).reshape(P, KC1 * S)
    buf[:S, MLP_CNT] = counts
    buf[:S, MLP_ID:MLP_ID + S] = np.eye(S, dtype=np.float32)
    return buf


def build_mlp_nc():
    """Stage-2 graph (1 core): reduced sums+counts -> probs via MLP+softmax."""
    f32 = mybir.dt.float32
    AF = mybir.ActivationFunctionType
    ALU = mybir.AluOpType

    nc = bacc.Bacc("TRN2", target_bir_lowering=False, debug=False,
                   num_devices=1)
    packed = nc.dram_tensor("packed", [P, MLP_W], f32, kind="ExternalInput").ap()
    probs = nc.dram_tensor("probs", [S, S], f32, kind="ExternalOutput").ap()

    with tile.TileContext(nc) as tc:
        with (
            tc.tile_pool(name="sb", bufs=1) as spool,
            tc.tile_pool(name="mm", bufs=1, space=bass.MemorySpace.PSUM) as mpool,
        ):
            pk = spool.tile([P, MLP_W], f32)
            # small operands (w2, biases, counts, sumsT, ident) first so the
            # matmul chain starts before the w1 slab finishes loading; w1
            # itself is split per contraction chunk across both HWDGE rings so
            # chunk-k matmuls fire as soon as chunk k lands
            nc.sync.dma_start(pk[:, MLP_W2:MLP_W], packed[:, MLP_W2:MLP_W])
            for k in range(KC1):
                eng = nc.scalar if k % 2 == 0 else nc.sync
                lo_, hi_ = MLP_W1 + k * H, MLP_W1 + (k + 1) * H
                eng.dma_start(pk[:, lo_:hi_], packed[:, lo_:hi_])
            # PE warm-up: ~3.4us of dummy matmuls while the DMAs land, so the
            # real chain runs at 2.4GHz instead of the 1.2GHz cold clock
            wsrc = spool.tile([1, 512], f32)
            nc.vector.memset(wsrc[:], 1.0)
            wps = mpool.tile([S, 512], f32, tag="warm")
            for w in range(2):
                nc.tensor.matmul(wps[:], wsrc[:1, 0:S], wsrc[:1, :],
                                 start=True, stop=True)
            ident_sb = pk[:S, MLP_ID:MLP_ID + S]
            w1_sb = pk[:, MLP_W1:MLP_W1 + KC1 * H].rearrange("p (k h) -> p k h",
                                                             k=KC1)
            w2_sb = pk[:, MLP_W2:MLP_W2 + KC2 * S].rearrange("p (k s) -> p k s",
                                                             k=KC2)
            tt_sb = pk[:, MLP_TT:MLP_TT + KC1 * S].rearrange("p (k s) -> p k s",
                                                             k=KC1)
            b1_sb = pk[0:1, MLP_B1:MLP_B1 + H]
            b2_sb = pk[0:1, MLP_B2:MLP_B2 + S]
            crow_sb = pk[0:1, MLP_CROW:MLP_CROW + S]
            cnt_sb = pk[:S, MLP_CNT:MLP_CNT + 1]
            ones_row = spool.tile([1, S], f32)
            nc.vector.memset(ones_row[:], 1.0)

            # ---- h = relu((sums @ W1 + counts (x) b1) / counts) ----
            recip = spool.tile([S, 1], f32)
            nc.vector.reciprocal(recip[:], cnt_sb)
            h_ps = mpool.tile([S, H], f32)
            for k in range(KC1):
                nc.tensor.matmul(h_ps[:], tt_sb[:, k, :], w1_sb[:, k, :],
                                 start=(k == 0), stop=False)
            nc.tensor.matmul(h_ps[:], crow_sb, b1_sb,
                             start=False, stop=True)
            h_sb = spool.tile([S, H], f32)
            nc.scalar.activation(h_sb[:], h_ps[:], AF.Relu, scale=recip[:, 0:1])

            # ---- logits = h @ W2 + b2 ----
            hT = spool.tile([P, KC2, S], f32)
            for k in range(KC2):
                pt2 = mpool.tile([P, S], f32, tag="pt")
                nc.tensor.transpose(pt2[:], h_sb[:, k * P:(k + 1) * P], ident_sb[:])
                nc.vector.tensor_copy(hT[:, k, :], pt2[:])
            l_ps = mpool.tile([S, S], f32)
            for k in range(KC2):
                nc.tensor.matmul(l_ps[:], hT[:, k, :], w2_sb[:, k, :],
                                 start=(k == 0), stop=False)
            nc.tensor.matmul(l_ps[:], ones_row[:1, :], b2_sb[:1, :],
                             start=False, stop=True)

            # ---- softmax over the free dim ----
            negmax = spool.tile([S, 1], f32)
            nc.vector.tensor_reduce(negmax[:], l_ps[:], axis=mybir.AxisListType.X,
                                    op=ALU.max, negate=True)
            e = spool.tile([S, S], f32)
            se = spool.tile([S, 1], f32)
            nc.scalar.activation(e[:], l_ps[:], AF.Exp, bias=negmax[:, 0:1],
                                 accum_out=se[:])
            rse = spool.tile([S, 1], f32)
            nc.vector.reciprocal(rse[:], se[:])
            pr = spool.tile([S, S], f32)
            nc.scalar.mul(pr[:], e[:], rse[:, 0:1])
            nc.sync.dma_start(probs[:], pr[:])

    nc.compile()
    return nc


def _get_nc(which, rows=ROWS, g=4):
    key = (which, rows, g)
    if key not in _nc_cache:
        if which == "main":
            _nc_cache[key] = build_main_nc(rows, g)
        else:
            _nc_cache[key] = build_mlp_nc()
    return _nc_cache[key]


def make_main_in_maps(x, subject_labels, rows=ROWS):
    """Shard x + labels into per-core stage-1 input maps (host side)."""
    bf = ml_dtypes.bfloat16
    n = x.shape[0]
    n_cores = n // rows
    T = rows // P
    xa = np.asarray(x, dtype=np.float32)
    hi = xa.astype(bf)
    lo = (xa - hi.astype(np.float32)).astype(bf)
    xhl = np.empty((n, 2, XW), bf)
    xhl[:, 0, :] = hi
    xhl[:, 1, :] = lo
    xhl = xhl.reshape(n_cores, rows, 2, XW)

    lab = np.asarray(subject_labels).astype(np.float32).reshape(n_cores, T, P)
    labt = np.ascontiguousarray(lab.transpose(0, 2, 1))  # [c, p, t]
    iota = np.ascontiguousarray(
        np.tile(np.arange(S, dtype=np.float32), (P, 1)))
    return [dict(xhl=xhl[c], labt=labt[c], iota=iota) for c in range(n_cores)]


def kernel(x, subject_labels, W1, b1, W2, b2):
    global LAST_EXEC_NS, LAST_MAIN_NS, LAST_MLP_NS, LAST_RESULTS
    x = np.asarray(x)
    subject_labels = np.asarray(subject_labels)
    kwargs = dict(trace=True) if PROFILE else {}

    # Stage 1: SPMD segment reduce over the 8 cores.
    nc1 = _get_nc("main")
    in_maps = make_main_in_maps(x, subject_labels)
    res1 = run_bass_kernel_spmd(nc1, in_maps, core_ids=list(range(N_CORES)),
                                **kwargs)
    # Gather/unshard: sum the 8 partial [32, XW] buffers.
    tot = np.zeros((S, D + 1), np.float64)
    for c in range(N_CORES):
        tot += res1.results[c]["part"]
    tot = np.ascontiguousarray(tot[:, :D + 1].astype(np.float32))

    # Stage 2: tiny MLP + softmax on one core.
    nc2 = _get_nc("mlp")
    mlp_in = dict(packed=pack_mlp_input(tot, W1, b1, W2, b2))
    res2 = run_bass_kernel_spmd(nc2, [mlp_in], core_ids=[0], **kwargs)

    LAST_MAIN_NS = res1.exec_time_ns
    LAST_MLP_NS = res2.exec_time_ns
    LAST_EXEC_NS = (None if res1.exec_time_ns is None or res2.exec_time_ns is None
                    else res1.exec_time_ns + res2.exec_time_ns)
    LAST_RESULTS = (res1, res2)
    probs = np.asarray(res2.results[0]["probs"], dtype=np.float32)
    unique_ids = np.arange(S, dtype=subject_labels.dtype)
    return probs, unique_ids


# revision 36
# speedup vs baseline: 1.0810x; 1.0810x over previous
"""Trainium2 Bass kernel for nn_DomainMapper (segment_reduce + tiny MLP).

Computation (matches the reference):
    sums[s]   = sum of x rows with label s          [32, 640]
    counts[s] = number of rows with label s         [32]
    feats     = sums / counts
    h         = relu(feats @ W1 + b1)               [32, 256]
    logits    = h @ W2 + b2                         [32, 32]
    probs     = softmax(logits, axis=-1)
    returns (probs, arange(32))

Strategy: data-parallel over 8 NeuronCores, two launches.

Stage 1 (SPMD x8): each core streams its 16384-row shard of x once (the
memory-bound part) and computes local segment sums + counts via one-hot
matmuls on the TensorEngine, writing a [32, 641] partial. On-chip
collectives were measured at 30-120us of exposed latency for this 82KB
payload (entry-barrier/launch-skew dominated), so the 8 partials are
gathered and summed on the host instead (8 x 82KB of shard glue).

Stage 2 (1 core): the reduced sums+counts go back to the device; the tiny
MLP + softmax runs in a second kernel and produces probs.

fp32 matmuls run at 1/4 PE rate on trn2, so x is shipped as a bf16 hi/lo
pair (x == hi + lo to ~16 mantissa bits): same 4 bytes/element of HBM
traffic as fp32, but the segment-sum matmuls run at full bf16 rate while
accumulating in fp32 PSUM. Counts are accumulated on the
VectorEngine (acc += onehot per subtile) and column-summed by one tiny
matmul at the end.
"""

import ml_dtypes
import numpy as np

import concourse.bass as bass
import concourse.bacc as bacc
import concourse.mybir as mybir
import concourse.tile as tile
from concourse.bass_utils import run_bass_kernel_spmd

N_CORES = 8
N, D, H, S = 131072, 640, 256, 32
XW = 640             # x hi/lo row: 2*640*2 = 2560 B, naturally 64B-aligned
                     # (counts come from a DVE one-hot accumulator instead of
                     # an appended ones column, so no pad bytes are read)
ROWS = N // N_CORES  # rows per core
P = 128              # partitions / rows per matmul subtile
KC1 = D // P         # 5 contraction chunks for feats @ W1
KC2 = H // P         # 2 contraction chunks for h @ W2

PROFILE = False
LAST_EXEC_NS = None
LAST_MAIN_NS = None
LAST_MLP_NS = None
LAST_RESULTS = None

_nc_cache = {}


def build_main_nc(rows=ROWS, g=4):
    """Stage-1 graph (SPMD x8): local segment sums + counts -> part[32, XW]."""
    T = rows // P            # number of 128-row subtiles
    assert T % g == 0
    f32 = mybir.dt.float32
    bf16 = mybir.dt.bfloat16
    ALU = mybir.AluOpType

    nc = bacc.Bacc("TRN2", target_bir_lowering=False, debug=False,
                   num_devices=N_CORES)

    xhl = nc.dram_tensor("xhl", [rows, 2, XW], bf16, kind="ExternalInput").ap()
    labt = nc.dram_tensor("labt", [P, T], f32, kind="ExternalInput").ap()
    iota = nc.dram_tensor("iota", [P, S], f32, kind="ExternalInput").ap()
    part_out = nc.dram_tensor("part", [S, D + 1], f32, kind="ExternalOutput").ap()

    with tile.TileContext(nc) as tc:
        with (
            tc.tile_pool(name="const", bufs=1) as cpool,
            tc.tile_pool(name="xload", bufs=2) as xpool,
            tc.tile_pool(name="oh", bufs=6) as ohpool,
            tc.tile_pool(name="acc", bufs=1, space=bass.MemorySpace.PSUM) as apool,
            tc.tile_pool(name="small", bufs=1) as spool,
        ):
            # labt/iota ride the scalar ring so the sync ring's first job is
            # x chunk 0 (first matmul fires ~3us earlier)
            labt_sb = cpool.tile([P, T], f32)
            nc.scalar.dma_start(labt_sb[:], labt[:])
            iota_sb = cpool.tile([P, S], f32)
            nc.scalar.dma_start(iota_sb[:], iota[:])
            acc_sb = cpool.tile([P, S], f32)   # per-partition one-hot sums
            nc.vector.memset(acc_sb[:], 0.0)
            ones_col = cpool.tile([P, 1], f32)
            nc.vector.memset(ones_col[:], 1.0)

            # ---- local segment sums: psum += onehot.T @ [x_hi; x_lo] ----
            psumA = apool.tile([S, 512], f32)       # x cols 0:512
            psumB = apool.tile([S, XW - 512], f32)  # x cols 512:640
            psumC = apool.tile([S, 1], f32)         # counts

            # Chunk plan: tiny leading DMAs so the PE starts ~20us earlier
            # (a big first DMA shares SDMA engines with the other queues and
            # completes late), then steady g-subtile groups.
            chunks = []
            lead = [1, 1, 2]
            if T > sum(lead) and (T - sum(lead)) % g == 0:
                t0 = 0
                for c in lead:
                    chunks.append((t0, c))
                    t0 += c
                while t0 < T:
                    chunks.append((t0, g))
                    t0 += g
            else:
                chunks = [(t0, g) for t0 in range(0, T, g)]

            xc = xhl.rearrange("(t p) two d -> p t two d", p=P)
            for ci, (t0, c) in enumerate(chunks):
                xt = xpool.tile([P, c, 2, XW], bf16, tag=f"xt{c}",
                                name=f"xt_{t0}", bufs=10 if c == g else 2)
                # alternate the two HWDGE rings (sync / scalar engines)
                dma_eng = nc.sync if ci % 2 == 0 else nc.scalar
                dma_eng.dma_start(xt[:], xc[:, t0:t0 + c])
                for j in range(c):
                    t = t0 + j
                    oh = ohpool.tile([P, S], bf16)
                    nc.vector.tensor_scalar(
                        oh[:], iota_sb[:], labt_sb[:, t:t + 1], None, ALU.is_equal)
                    nc.vector.tensor_tensor(acc_sb[:], acc_sb[:], oh[:], ALU.add)
                    first = (t == 0)
                    last = (t == T - 1)
                    # hi pass then lo pass share the same stationary one-hot
                    nc.tensor.matmul(psumA[:], oh[:], xt[:, j, 0, 0:512],
                                     start=first, stop=False)
                    nc.tensor.matmul(psumB[:], oh[:], xt[:, j, 0, 512:XW],
                                     start=first, stop=False)
                    nc.tensor.matmul(psumA[:], oh[:], xt[:, j, 1, 0:512],
                                     start=False, stop=last)
                    nc.tensor.matmul(psumB[:], oh[:], xt[:, j, 1, 512:XW],
                                     start=False, stop=last)

            # counts[s] = sum_p acc[p, s] via a single 1-column matmul
            nc.tensor.matmul(psumC[:], acc_sb[:], ones_col[:])
            part = spool.tile([S, D + 1], f32)
            nc.vector.tensor_copy(part[:, 0:512], psumA[:])
            nc.vector.tensor_copy(part[:, 512:D], psumB[:])
            nc.vector.tensor_copy(part[:, D:D + 1], psumC[:])
            nc.sync.dma_start(part_out[:], part[:])

    nc.compile()
    return nc


# Stage-2 packed input layout: one [128, MLP_W] f32 buffer holding everything.
# sums are packed TRANSPOSED ([p, k, s] = sums[s, k*128+p]) so the feats @ W1
# contraction needs no on-device transposes; 1/counts is applied after that
# matmul (diag(1/c) @ (sums@W1)) via the ReLU activation's per-partition
# scale, with counts (x) b1 added in PSUM by a counts-row matmul so the bias
# comes out right: relu((sums@W1 + c (x) b1) / c) == relu(feats@W1 + b1).
MLP_W1 = 0                       # [128, KC1*H]
MLP_W2 = MLP_W1 + KC1 * H        # [128, KC2*S]
MLP_B1 = MLP_W2 + KC2 * S        # [1, H] (partition 0)
MLP_B2 = MLP_B1 + H              # [1, S] (partition 0)
MLP_CROW = MLP_B2 + S            # [1, S] counts row (partition 0)
MLP_TT = MLP_CROW + S            # [128, KC1*S] sums transposed
MLP_CNT = MLP_TT + KC1 * S       # [32, 1] counts column
MLP_ID = MLP_CNT + 1             # [32, S] identity (partitions 0:32)
MLP_W = MLP_ID + S


def pack_mlp_input(tot, W1, b1, W2, b2):
    buf = np.zeros((P, MLP_W), np.float32)
    w1 = np.asarray(W1, dtype=np.float32).reshape(KC1, P, H)
    buf[:, MLP_W1:MLP_W1 + KC1 * H] = w1.transpose(1, 0, 2).reshape(P, KC1 * H)
    w2 = np.asarray(W2, dtype=np.float32).reshape(KC2, P, S)
    buf[:, MLP_W2:MLP_W2 + KC2 * S] = w2.transpose(1, 0, 2).reshape(P, KC2 * S)
    buf[0, MLP_B1:MLP_B1 + H] = np.asarray(b1, dtype=np.float32).ravel()
    buf[0, MLP_B2:MLP_B2 + S] = np.asarray(b2, dtype=np.float32).ravel()
    counts = tot[:, D]
    buf[0, MLP_CROW:MLP_CROW + S] = counts
    tt = tot[:, :D].T.reshape(KC1, P, S)  # [k, p, s] = sums[s, k*128+p]
    buf[:, MLP_TT:MLP_TT + KC1 * S] = tt.transpose(1, 0, 2).reshape(P, KC1 * S)
    buf[:S, MLP_CNT] = counts
    buf[:S, MLP_ID:MLP_ID + S] = np.eye(S, dtype=np.float32)
    return buf


def build_mlp_nc():
    """Stage-2 graph (1 core): reduced sums+counts -> probs via MLP+softmax."""
    f32 = mybir.dt.float32
    AF = mybir.ActivationFunctionType
    ALU = mybir.AluOpType

    nc = bacc.Bacc("TRN2", target_bir_lowering=False, debug=False,
                   num_devices=1)
    packed = nc.dram_tensor("packed", [P, MLP_W], f32, kind="ExternalInput").ap()
    probs = nc.dram_tensor("probs", [S, S], f32, kind="ExternalOutput").ap()

    with tile.TileContext(nc) as tc:
        with (
            tc.tile_pool(name="sb", bufs=1) as spool,
            tc.tile_pool(name="mm", bufs=1, space=bass.MemorySpace.PSUM) as mpool,
        ):
            pk = spool.tile([P, MLP_W], f32)
            # small operands (w2, biases, counts, sumsT, ident) first so the
            # matmul chain starts before the w1 slab finishes loading; w1
            # itself is split per contraction chunk across both HWDGE rings so
            # chunk-k matmuls fire as soon as chunk k lands
            nc.sync.dma_start(pk[:, MLP_W2:MLP_W], packed[:, MLP_W2:MLP_W])
            for k in range(KC1):
                eng = nc.scalar if k % 2 == 0 else nc.sync
                lo_, hi_ = MLP_W1 + k * H, MLP_W1 + (k + 1) * H
                eng.dma_start(pk[:, lo_:hi_], packed[:, lo_:hi_])
            # PE warm-up: ~3.4us of dummy matmuls while the DMAs land, so the
            # real chain runs at 2.4GHz instead of the 1.2GHz cold clock
            wsrc = spool.tile([1, 512], f32)
            nc.vector.memset(wsrc[:], 1.0)
            wps = mpool.tile([S, 512], f32, tag="warm")
            # many short warm-up matmuls instead of two long ones: same ~3us
            # of HAM-warming busy time, but the real chain's first matmul only
            # queues behind <=0.5us of warm-up once its data lands
            for w in range(4):
                nc.tensor.matmul(wps[:, 0:128], wsrc[:1, 0:S], wsrc[:1, 0:128],
                                 start=True, stop=True)
            ident_sb = pk[:S, MLP_ID:MLP_ID + S]
            w1_sb = pk[:, MLP_W1:MLP_W1 + KC1 * H].rearrange("p (k h) -> p k h",
                                                             k=KC1)
            w2_sb = pk[:, MLP_W2:MLP_W2 + KC2 * S].rearrange("p (k s) -> p k s",
                                                             k=KC2)
            tt_sb = pk[:, MLP_TT:MLP_TT + KC1 * S].rearrange("p (k s) -> p k s",
                                                             k=KC1)
            b1_sb = pk[0:1, MLP_B1:MLP_B1 + H]
            b2_sb = pk[0:1, MLP_B2:MLP_B2 + S]
            crow_sb = pk[0:1, MLP_CROW:MLP_CROW + S]
            cnt_sb = pk[:S, MLP_CNT:MLP_CNT + 1]
            ones_row = spool.tile([1, S], f32)
            nc.vector.memset(ones_row[:], 1.0)

            # ---- h = relu((sums @ W1 + counts (x) b1) / counts) ----
            recip = spool.tile([S, 1], f32)
            nc.vector.reciprocal(recip[:], cnt_sb)
            h_ps = mpool.tile([S, H], f32)
            for k in range(KC1):
                nc.tensor.matmul(h_ps[:], tt_sb[:, k, :], w1_sb[:, k, :],
                                 start=(k == 0), stop=False)
            nc.tensor.matmul(h_ps[:], crow_sb, b1_sb,
                             start=False, stop=True)
            h_sb = spool.tile([S, H], f32)
            nc.scalar.activation(h_sb[:], h_ps[:], AF.Relu, scale=recip[:, 0:1])

            # ---- logits = h @ W2 + b2 ----
            hT = spool.tile([P, KC2, S], f32)
            for k in range(KC2):
                pt2 = mpool.tile([P, S], f32, tag="pt")
                nc.tensor.transpose(pt2[:], h_sb[:, k * P:(k + 1) * P], ident_sb[:])
                nc.vector.tensor_copy(hT[:, k, :], pt2[:])
            l_ps = mpool.tile([S, S], f32)
            for k in range(KC2):
                nc.tensor.matmul(l_ps[:], hT[:, k, :], w2_sb[:, k, :],
                                 start=(k == 0), stop=False)
            nc.tensor.matmul(l_ps[:], ones_row[:1, :], b2_sb[:1, :],
                             start=False, stop=True)

            # ---- softmax over the free dim ----
            negmax = spool.tile([S, 1], f32)
            nc.vector.tensor_reduce(negmax[:], l_ps[:], axis=mybir.AxisListType.X,
                                    op=ALU.max, negate=True)
            e = spool.tile([S, S], f32)
            se = spool.tile([S, 1], f32)
            nc.scalar.activation(e[:], l_ps[:], AF.Exp, bias=negmax[:, 0:1],
                                 accum_out=se[:])
            rse = spool.tile([S, 1], f32)
            nc.vector.reciprocal(rse[:], se[:])
            pr = spool.tile([S, S], f32)
            nc.scalar.mul(pr[:], e[:], rse[:, 0:1])
            nc.sync.dma_start(probs[:], pr[:])

    nc.compile()
    return nc


def _get_nc(which, rows=ROWS, g=4):
    key = (which, rows, g)
    if key not in _nc_cache:
        if which == "main":
            _nc_cache[key] = build_main_nc(rows, g)
        else:
            _nc_cache[key] = build_mlp_nc()
    return _nc_cache[key]


def make_main_in_maps(x, subject_labels, rows=ROWS):
    """Shard x + labels into per-core stage-1 input maps (host side)."""
    bf = ml_dtypes.bfloat16
    n = x.shape[0]
    n_cores = n // rows
    T = rows // P
    xa = np.asarray(x, dtype=np.float32)
    hi = xa.astype(bf)
    lo = (xa - hi.astype(np.float32)).astype(bf)
    xhl = np.empty((n, 2, XW), bf)
    xhl[:, 0, :] = hi
    xhl[:, 1, :] = lo
    xhl = xhl.reshape(n_cores, rows, 2, XW)

    lab = np.asarray(subject_labels).astype(np.float32).reshape(n_cores, T, P)
    labt = np.ascontiguousarray(lab.transpose(0, 2, 1))  # [c, p, t]
    iota = np.ascontiguousarray(
        np.tile(np.arange(S, dtype=np.float32), (P, 1)))
    return [dict(xhl=xhl[c], labt=labt[c], iota=iota) for c in range(n_cores)]


def kernel(x, subject_labels, W1, b1, W2, b2):
    global LAST_EXEC_NS, LAST_MAIN_NS, LAST_MLP_NS, LAST_RESULTS
    x = np.asarray(x)
    subject_labels = np.asarray(subject_labels)
    kwargs = dict(trace=True) if PROFILE else {}

    # Stage 1: SPMD segment reduce over the 8 cores.
    nc1 = _get_nc("main")
    in_maps = make_main_in_maps(x, subject_labels)
    res1 = run_bass_kernel_spmd(nc1, in_maps, core_ids=list(range(N_CORES)),
                                **kwargs)
    # Gather/unshard: sum the 8 partial [32, XW] buffers.
    tot = np.zeros((S, D + 1), np.float64)
    for c in range(N_CORES):
        tot += res1.results[c]["part"]
    tot = np.ascontiguousarray(tot[:, :D + 1].astype(np.float32))

    # Stage 2: tiny MLP + softmax on one core.
    nc2 = _get_nc("mlp")
    mlp_in = dict(packed=pack_mlp_input(tot, W1, b1, W2, b2))
    res2 = run_bass_kernel_spmd(nc2, [mlp_in], core_ids=[0], **kwargs)

    LAST_MAIN_NS = res1.exec_time_ns
    LAST_MLP_NS = res2.exec_time_ns
    LAST_EXEC_NS = (None if res1.exec_time_ns is None or res2.exec_time_ns is None
                    else res1.exec_time_ns + res2.exec_time_ns)
    LAST_RESULTS = (res1, res2)
    probs = np.asarray(res2.results[0]["probs"], dtype=np.float32)
    unique_ids = np.arange(S, dtype=subject_labels.dtype)
    return probs, unique_ids
